# revision 25
# baseline (speedup 1.0000x reference)
"""Trainium2 Bass kernel for nn_MultiHeadAttention (B=2, S=2048, D=1024, H=16).

Sharding: 8 cores = 2 batch groups x 4 cores. Core c handles batch c//4 and
heads 4*(c%4) .. 4*(c%4)+4 (CHD=256 head-dims). Each core computes Q/K/V
projections for its batch+heads, transposed-layout attention (softmax
denominators via a ones-augmented V column), and a partial output projection
over its 256 head-dims. Host sums the 4 bf16 partials per batch in f32.

All tensors bf16 (PSUM accumulation f32). Inputs are pre-arranged on the host
into [128-partition, flat] layouts so every DMA is a contiguous run per
partition (cheap descriptors). The scalar-engine exp is the pacing engine
(~147us); the scores+exp stream runs up to two head-pair groups ahead of the
attn@V stream (pt tiles buffered in SBUF) so exp also saturates during K/V
projection; Q/out projections drain as fillers inside the attention stream.
"""

from collections import deque

import numpy as np

B, S, D, H = 2, 2048, 1024, 16
HD = D // H          # 64
NCORES = 8
HPC = 4              # heads per core
CHD = HPC * HD       # 256 head-dims per core
TOK = S              # tokens per core (one batch)
QW = 512             # query window
NQW = TOK // QW      # 4 windows
NKT = TOK // 128     # 16 key tiles
CB = 256             # K/V projection token-block
NCB = TOK // CB      # 8 blocks
SCALE = 1.0 / np.sqrt(np.float32(D))  # 1/32

_PROG = None
_LAST_IN_MAPS = None


def _build(debug=False):
    from contextlib import ExitStack

    import concourse.bass as bass
    import concourse.tile as tile
    from concourse import bacc, mybir

    BF16 = mybir.dt.bfloat16
    F32 = mybir.dt.float32
    EXP = mybir.ActivationFunctionType.Exp

    nc = bacc.Bacc("TRN2", target_bir_lowering=False, debug=False,
                   num_devices=NCORES)

    # host-pre-arranged flat layouts: contiguous per-partition runs
    xqW = nc.dram_tensor("xqW", [128, NQW * 8 * QW], BF16, kind="ExternalInput").ap()
    xkB = nc.dram_tensor("xkB", [128, NCB * 8 * CB], BF16, kind="ExternalInput").ap()
    xvB = nc.dram_tensor("xvB", [128, NCB * 8 * CB], BF16, kind="ExternalInput").ap()
    wqF = nc.dram_tensor("wqF", [128, 8 * CHD], BF16, kind="ExternalInput").ap()
    wkF = nc.dram_tensor("wkF", [128, 8 * CHD], BF16, kind="ExternalInput").ap()
    wvF = nc.dram_tensor("wvF", [128, 8 * CHD], BF16, kind="ExternalInput").ap()
    woTs = nc.dram_tensor("woTs", [CHD, D], BF16, kind="ExternalInput").ap()
    pout = nc.dram_tensor("pout", [TOK, D], BF16, kind="ExternalOutput").ap()
    if debug:
        dbg = {
            "dKT": nc.dram_tensor("dKT", [2, 128, TOK], BF16, kind="ExternalOutput").ap(),
            "dvnat": nc.dram_tensor("dvnat", [128, NKT * 260], BF16, kind="ExternalOutput").ap(),
            "dqt": nc.dram_tensor("dqt", [2, 128, QW], BF16, kind="ExternalOutput").ap(),
            "dpt": nc.dram_tensor("dpt", [128, 2 * QW], BF16, kind="ExternalOutput").ap(),
            "deAB": nc.dram_tensor("deAB", [2, 65, QW], F32, kind="ExternalOutput").ap(),
            "drr": nc.dram_tensor("drr", [2, QW], F32, kind="ExternalOutput").ap(),
            "dbc": nc.dram_tensor("dbc", [2, 64, QW], F32, kind="ExternalOutput").ap(),
            "dctx": nc.dram_tensor("dctx", [2, 128, TOK], BF16, kind="ExternalOutput").ap(),
        }

    with tile.TileContext(nc) as tc, ExitStack() as ctx:
        const = ctx.enter_context(tc.tile_pool(name="const", bufs=1))
        wq_sb = const.tile([128, 8 * CHD], BF16, tag="wq")
        wk_sb = const.tile([128, 8 * CHD], BF16, tag="wk")
        wv_sb = const.tile([128, 8 * CHD], BF16, tag="wv")
        wo_sb = [const.tile([128, D], BF16, tag=f"wo{p}", name=f"wo{p}")
                 for p in range(2)]

        def wsl(t, ko, lo, hi):
            return t[:, ko * CHD + lo:ko * CHD + hi]

        # startup DMA order: K-proj deps first (wk, xbk0), then Q (wq, qx0)
        nc.sync.dma_start(out=wk_sb, in_=wkF)
        xblk = ctx.enter_context(tc.tile_pool(name="xblk", bufs=4))
        xbk0 = xblk.tile([128, 8 * CB], BF16, tag="xb", name="xbk0")
        nc.sync.dma_start(out=xbk0, in_=xkB[:, 0:8 * CB])
        nc.sync.dma_start(out=wq_sb, in_=wqF)
        qx_pool = ctx.enter_context(tc.tile_pool(name="qx", bufs=2))
        qx0 = qx_pool.tile([128, 8 * QW], BF16, tag="qx", name="qx0")
        nc.sync.dma_start(out=qx0, in_=xqW[:, 0:8 * QW])

        # warm the exp table early
        warm = const.tile([1, 8], F32)
        nc.vector.memset(warm, 0.0)
        nc.scalar.activation(out=warm, in_=warm, func=EXP)

        # persistent attention operands
        kqt = ctx.enter_context(tc.tile_pool(name="kqt", bufs=1))
        KT = [kqt.tile([128, TOK], BF16, tag=f"kt{m}", name=f"KT{m}")
              for m in range(2)]
        vnat = kqt.tile([128, NKT, HPC * (HD + 1)], BF16, tag="vnat")
        ctxP = [kqt.tile([128, TOK], BF16, tag=f"ctxP{p}", name=f"ctxP{p}")
                for p in range(2)]
        ones16 = const.tile([128, NKT], BF16)
        nc.vector.memset(ones16, 1.0)
        ones64f = const.tile([1, 64], F32, tag="ones64f")
        nc.vector.memset(ones64f, 1.0)
        ones64r = const.tile([1, 64], mybir.dt.float32r, tag="ones64r")
        nc.vector.tensor_copy(ones64r[:], ones64f[:])
        for h in range(HPC):
            nc.vector.tensor_copy(vnat[:, :, h * 65 + 64], ones16[:])

        # PSUM: sc 2x[128,1024]f32 (4 banks) + cop 2x[65,512] (2) + pp 2x[128,512] (2)
        sc_ps = ctx.enter_context(tc.tile_pool(name="sc_ps", bufs=2, space="PSUM"))
        cop = ctx.enter_context(tc.tile_pool(name="cop", bufs=2, space="PSUM"))
        pp = ctx.enter_context(tc.tile_pool(name="pp", bufs=2, space="PSUM"))

        qt_pool = ctx.enter_context(tc.tile_pool(name="qtw", bufs=4))
        pt_pool = ctx.enter_context(tc.tile_pool(name="ptp", bufs=36))
        nrm = ctx.enter_context(tc.tile_pool(name="nrm", bufs=2))
        rdp = ctx.enter_context(tc.tile_pool(name="rdp", bufs=2, space="DRAM"))
        oev = ctx.enter_context(tc.tile_pool(name="oev", bufs=3))

        qts = {}   # window -> [qt_pair0, qt_pair1];  (w, "x") -> staged qx
        fillers = deque()

        def drain(n):
            for _ in range(min(n, len(fillers))):
                fillers.popleft()()

        def flush():
            drain(len(fillers))

        # ---- Q projection (window w) as units ----
        def emit_qproj_units(w, dma_now=False):
            def u_dma():
                qxb = qx_pool.tile([128, 8 * QW], BF16, tag="qx", name=f"qx{w}")
                nc.sync.dma_start(out=qxb, in_=xqW[:, w * 8 * QW:(w + 1) * 8 * QW])
                qts[(w, "x")] = qxb

            def mk_mm(m):
                def u_mm():
                    qp = pp.tile([128, QW], F32, tag="pp", name=f"qp{w}_{m}")
                    qxb = qts[(w, "x")]
                    for ko in range(8):
                        nc.tensor.matmul(
                            qp[:], wsl(wq_sb, ko, m * 128, (m + 1) * 128),
                            qxb[:, ko * QW:(ko + 1) * QW],
                            start=(ko == 0), stop=(ko == 7))
                    qt = qt_pool.tile([128, QW], BF16, tag="qt", name=f"qt{w}_{m}")
                    nc.vector.tensor_copy(qt[:], qp[:])
                    qts.setdefault(w, [None, None])[m] = qt
                return u_mm

            if w == 0:
                qts[(0, "x")] = qx0
                for u in (mk_mm(0), mk_mm(1)):
                    u()
            elif dma_now:
                u_dma()
                fillers.extend([mk_mm(0), mk_mm(1)])
            else:
                fillers.extend([u_dma, mk_mm(0), mk_mm(1)])

        # ---- output projection (window w) as units ----
        def emit_outproj_units(w, alt_pool=False):
            q0 = w * QW
            for tt in range(QW // 128):
                t0 = q0 + tt * 128
                for et in range(2):
                    box = {}
                    use_sc = alt_pool and (tt * 2 + et) % 2 == 1

                    def u_mm(t0=t0, et=et, box=box, use_sc=use_sc):
                        if use_sc:
                            pol = sc_ps.tile([128, 2 * QW], F32, tag="sc",
                                             name=f"po{t0}_{et}")
                            po = pol[:, 0:512]
                        else:
                            po = pp.tile([128, 512], F32, tag="pp",
                                         name=f"po{t0}_{et}")
                        for p in range(2):
                            nc.tensor.matmul(
                                po[:], ctxP[p][:, t0:t0 + 128],
                                wo_sb[p][:, et * 512:(et + 1) * 512],
                                start=(p == 0), stop=(p == 1))
                        box["po"] = po

                    def u_ev(t0=t0, et=et, box=box):
                        ot = oev.tile([128, 512], BF16, tag="ot")
                        nc.vector.tensor_copy(ot[:], box["po"][:])
                        nc.sync.dma_start(
                            out=pout[t0:t0 + 128, et * 512:(et + 1) * 512],
                            in_=ot[:])

                    fillers.append(u_mm)
                    fillers.append(u_ev)

        # ---- attention pieces ----
        def scores_exp(w, p, kt):
            qtp = qts[w][p]
            sc = sc_ps.tile([128, 2 * QW], F32, tag="sc", name=f"sc{w}_{p}_{kt}")
            nc.tensor.matmul(
                sc[:, 0:QW], KT[p][0:64, kt * 128:(kt + 1) * 128],
                qtp[0:64, :], start=True, stop=True, tile_position=(0, 0))
            nc.tensor.matmul(
                sc[:, QW:2 * QW], KT[p][64:128, kt * 128:(kt + 1) * 128],
                qtp[64:128, :], start=True, stop=True, tile_position=(64, 0))
            pt = pt_pool.tile([128, 2 * QW], BF16, tag="pt")
            nc.scalar.activation(out=pt[:], in_=sc[:], func=EXP,
                                 scale=float(SCALE))
            if debug and (w, p, kt) == (0, 0, 0):
                nc.sync.dma_start(out=dbg["dpt"], in_=pt[:])
            return pt

        def attn_v(p, kt, pt, cA, cB):
            hA, hB = 2 * p, 2 * p + 1
            nc.tensor.matmul(
                cA[:], vnat[:, kt, hA * 65:hA * 65 + 65], pt[:, 0:QW],
                start=(kt == 0), stop=(kt == NKT - 1))
            nc.tensor.matmul(
                cB[:], vnat[:, kt, hB * 65:hB * 65 + 65], pt[:, QW:2 * QW],
                start=(kt == 0), stop=(kt == NKT - 1))

        # ---- softmax normalize: part A frees PSUM accumulators fast; ----
        # ---- part B (DRAM-broadcast + muls) runs later as a filler. ----
        def normalize_a(w, p, cA, cB):
            eA = nrm.tile([65, QW], F32, tag="eA")
            eB = nrm.tile([65, QW], F32, tag="eB")
            nc.vector.tensor_copy(eA[:], cA[:])
            nc.vector.tensor_copy(eB[:], cB[:])
            rsA = nrm.tile([1, QW], F32, tag="rsA")
            rsB = nrm.tile([1, QW], F32, tag="rsB")
            nc.vector.tensor_copy(rsA[:], eA[64:65, :])
            nc.vector.tensor_copy(rsB[:], eB[64:65, :])
            rrA = nrm.tile([1, QW], F32, tag="rrA")
            rrB = nrm.tile([1, QW], F32, tag="rrB")
            nc.vector.reciprocal_approx_fast(rrA[:], rsA[:])
            nc.vector.reciprocal_approx_fast(rrB[:], rsB[:])
            if debug and (w, p) == (0, 0):
                nc.sync.dma_start(out=dbg["deAB"][0], in_=eA[:])
                nc.sync.dma_start(out=dbg["deAB"][1], in_=eB[:])
                nc.sync.dma_start(out=dbg["drr"][0:1], in_=rrA[:])
                nc.sync.dma_start(out=dbg["drr"][1:2], in_=rrB[:])
            return eA, eB, rrA, rrB

        def normalize_b(w, p, eA, eB, rrA, rrB):
            q0 = w * QW
            rden = rdp.tile([2, QW], F32, tag="rden")
            nc.gpsimd.dma_start(out=rden[0:1, :], in_=rrA[:])
            nc.gpsimd.dma_start(out=rden[1:2, :], in_=rrB[:])
            bcA = nrm.tile([64, QW], F32, tag="bcA")
            bcB = nrm.tile([64, QW], F32, tag="bcB")
            nc.gpsimd.dma_start(
                out=bcA, in_=bass.AP(tensor=rden.tensor, offset=rden.offset,
                                     ap=[[0, 64], [1, QW]]))
            nc.gpsimd.dma_start(
                out=bcB, in_=bass.AP(tensor=rden.tensor,
                                     offset=rden.offset + QW,
                                     ap=[[0, 64], [1, QW]]))
            if debug and (w, p) == (0, 0):
                nc.sync.dma_start(out=dbg["dbc"][0], in_=bcA[:])
                nc.sync.dma_start(out=dbg["dbc"][1], in_=bcB[:])
            nc.vector.tensor_mul(ctxP[p][0:64, q0:q0 + QW], eA[0:64, :],
                                 bcA[:])
            scb = nrm.tile([64, QW], BF16, tag="scb")
            nc.vector.tensor_mul(scb[:], eB[0:64, :], bcB[:])
            nc.gpsimd.dma_start(out=ctxP[p][64:128, q0:q0 + QW], in_=scb[:])

        def normalize_b_pe(w, p, eA, eB, rrA, rrB):
            q0 = w * QW
            rrAr = nrm.tile([1, QW], mybir.dt.float32r, tag="rrAr")
            rrBr = nrm.tile([1, QW], mybir.dt.float32r, tag="rrBr")
            nc.vector.tensor_copy(rrAr[:], rrA[:])
            nc.vector.tensor_copy(rrBr[:], rrB[:])
            bc = sc_ps.tile([128, 2 * QW], F32, tag="sc", name=f"bcp{w}_{p}")
            nc.tensor.matmul(bc[0:64, 0:QW], ones64r[:], rrAr[:],
                             start=True, stop=True)
            nc.tensor.matmul(bc[0:64, QW:2 * QW], ones64r[:], rrBr[:],
                             start=True, stop=True)
            nc.vector.tensor_mul(ctxP[p][0:64, q0:q0 + QW], eA[0:64, :],
                                 bc[0:64, 0:QW])
            scb = nrm.tile([64, QW], BF16, tag="scb")
            nc.vector.tensor_mul(scb[:], eB[0:64, :], bc[0:64, QW:2 * QW])
            nc.gpsimd.dma_start(out=ctxP[p][64:128, q0:q0 + QW], in_=scb[:])

        # ---------------- schedule ----------------
        # PE warm-keeper: dependency-free tiny matmuls run while the first
        # input DMAs stream in, keeping the HAM clock-gate at full rate.
        dwp = pp.tile([128, 512], F32, tag="pp", name="dwp")
        for i in range(80):
            nc.tensor.matmul(dwp[0:16, 0:16], ones16[:, 0:16], ones16[:, 0:16],
                             start=True, stop=True)
        emit_qproj_units(0)

        grp = [(w, p) for w in range(NQW) for p in range(2)]
        pt_store = {}          # (group_idx, kt) -> pt tile

        attn_state = {"g": 0, "kt": 0, "cA": None, "cB": None}

        def emit_attn_step():
            g, kt = attn_state["g"], attn_state["kt"]
            if g >= len(grp) or (g, kt) not in pt_store:
                return False
            w, p = grp[g]
            if kt == 0:
                attn_state["cA"] = cop.tile([65, QW], F32, tag="ctx",
                                            name=f"cA_{w}_{p}")
                attn_state["cB"] = cop.tile([65, QW], F32, tag="ctx",
                                            name=f"cB_{w}_{p}")
            pt = pt_store.pop((g, kt))
            attn_v(p, kt, pt, attn_state["cA"], attn_state["cB"])
            if kt == NKT - 1:
                ns = normalize_a(w, p, attn_state["cA"], attn_state["cB"])
                if g == len(grp) - 1:
                    normalize_b_pe(w, p, *ns)
                    dwp2 = pp.tile([128, 512], F32, tag="pp", name="dwp2")
                    for _ in range(60):
                        nc.tensor.matmul(dwp2[0:16, 0:16], ones16[:, 0:16],
                                         ones16[:, 0:16], start=True, stop=True)
                else:
                    fillers.append(
                        lambda w=w, p=p, ns=ns: normalize_b(w, p, *ns))
                if p == 1:
                    emit_outproj_units(w, alt_pool=(w == NQW - 1))
                attn_state["g"] += 1
                attn_state["kt"] = 0
            else:
                attn_state["kt"] = kt + 1
            return True

        # ---- phase 0: K/V projection blocks + scores/exp for window-0 ----
        emit_qproj_units(1, dma_now=True)
        for blk in range(NCB):
            c0 = blk * CB
            if blk > 0:
                xbk = xblk.tile([128, 8 * CB], BF16, tag="xb", name=f"xbk{blk}")
                nc.sync.dma_start(out=xbk,
                                  in_=xkB[:, blk * 8 * CB:(blk + 1) * 8 * CB])
            else:
                xbk = xbk0
                nc.sync.dma_start(out=wv_sb, in_=wvF)
            xbv = xblk.tile([128, 8 * CB], BF16, tag="xb", name=f"xbv{blk}")
            nc.sync.dma_start(out=xbv,
                              in_=xvB[:, blk * 8 * CB:(blk + 1) * 8 * CB])
            if blk == 6:
                for p_ in range(2):
                    nc.sync.dma_start(out=wo_sb[p_],
                                      in_=woTs[p_ * 128:(p_ + 1) * 128, :])
            psk = pp.tile([128, 2 * CB], F32, tag="pp", name=f"psk{blk}")
            for m in range(2):
                for ko in range(8):
                    nc.tensor.matmul(
                        psk[:, m * CB:(m + 1) * CB],
                        wsl(wk_sb, ko, m * 128, (m + 1) * 128),
                        xbk[:, ko * CB:(ko + 1) * CB],
                        start=(ko == 0), stop=(ko == 7))
            for m in range(2):
                nc.vector.tensor_copy(
                    KT[m][:, c0:c0 + CB], psk[:, m * CB:(m + 1) * CB])
            # scores+exp for both window-0 head pairs on this block's key
            # tiles; V-proj matmuls cover the exp latency.
            for kt in (2 * blk, 2 * blk + 1):
                pt_store[(0, kt)] = scores_exp(0, 0, kt)
            for tt in range(2):
                bi = 2 * blk + tt
                pv = pp.tile([128, CHD], F32, tag="pp", name=f"pv{bi}")
                for ko in range(8):
                    nc.tensor.matmul(
                        pv[:], xbv[:, ko * CB + tt * 128:ko * CB + (tt + 1) * 128],
                        wsl(wv_sb, ko, 0, CHD), start=(ko == 0), stop=(ko == 7))
                nc.vector.tensor_copy(
                    vnat[:, bi, :].rearrange("p (h c) -> p h c", c=65)[:, :, 0:64],
                    pv[:].rearrange("p (h c) -> p h c", c=64))
            for kt in (2 * blk, 2 * blk + 1):
                pt_store[(1, kt)] = scores_exp(0, 1, kt)
            drain(2)

        # ---- phase 1: exp stream 2 groups ahead; attn catches up ----
        for eg in range(2, len(grp) + 2):
            if eg < len(grp):
                w, p = grp[eg]
                if p == 0 and w + 1 < NQW:
                    emit_qproj_units(w + 1)
            for kt in range(NKT):
                if eg < len(grp):
                    pt_store[(eg, kt)] = scores_exp(grp[eg][0], grp[eg][1], kt)
                nsteps = 2 if len(pt_store) > 3 else 1
                for _ in range(nsteps):
                    emit_attn_step()
                # near attn-group boundaries keep the vector queue shallow so
                # the next group's PSUM accumulators free up promptly
                akt = attn_state["kt"]
                endgame = (attn_state["g"] == len(grp) - 1 and akt > 8)
                if 2 <= akt <= 12 and not endgame:
                    drain(1 if len(fillers) < 12 else 2)
        while emit_attn_step():
            if attn_state["g"] < len(grp) - 1 or attn_state["kt"] <= 8:
                drain(1)
        flush()
        if debug:
            for m in range(2):
                nc.sync.dma_start(out=dbg["dKT"][m], in_=KT[m][:])
            nc.sync.dma_start(
                out=dbg["dvnat"],
                in_=vnat[:].rearrange("p a b -> p (a b)"))
            for p_ in range(2):
                nc.sync.dma_start(out=dbg["dqt"][p_], in_=qts[0][p_][:])
                nc.sync.dma_start(out=dbg["dctx"][p_], in_=ctxP[p_][:])

    nc.compile()
    return nc


def kernel(query, key, value, Wq, Wk, Wv, Wo, debug=False):
    global _PROG, _LAST_IN_MAPS
    import ml_dtypes
    from concourse.bass_utils import run_bass_kernel_spmd

    if _PROG is None or debug:
        _PROG = _build(debug=debug)
    nc = _PROG
    if debug:
        _PROG = None

    bf16 = ml_dtypes.bfloat16
    q2 = np.asarray(query, dtype=np.float32).reshape(B, S, D)
    k2 = np.asarray(key, dtype=np.float32).reshape(B, S, D)
    v2 = np.asarray(value, dtype=np.float32).reshape(B, S, D)
    Wq = np.asarray(Wq, dtype=np.float32)
    Wk = np.asarray(Wk, dtype=np.float32)
    Wv = np.asarray(Wv, dtype=np.float32)
    Wo = np.asarray(Wo, dtype=np.float32)

    def xq_arrange(x):  # [S, D] -> [128, NQW*8*QW] window-major
        a = x.T.reshape(8, 128, S).transpose(1, 0, 2)          # ki, ko, t
        a = a.reshape(128, 8, NQW, QW).transpose(0, 2, 1, 3)   # ki, w, ko, t
        return np.ascontiguousarray(a.reshape(128, NQW * 8 * QW)).astype(bf16)

    def xkv_arrange(x):  # [S, D] -> [128, NCB*8*CB] block-major
        a = x.T.reshape(8, 128, S).transpose(1, 0, 2)
        a = a.reshape(128, 8, NCB, CB).transpose(0, 2, 1, 3)   # ki, blk, ko, t
        return np.ascontiguousarray(a.reshape(128, NCB * 8 * CB)).astype(bf16)

    def w_arrange(Wsl):  # [CHD, D] slice -> [128, 8*CHD]
        a = Wsl.T.reshape(8, 128, CHD).transpose(1, 0, 2)      # ki, ko, m
        return np.ascontiguousarray(a.reshape(128, 8 * CHD)).astype(bf16)

    xq = {b: xq_arrange(q2[b]) for b in range(B)}
    xk = {b: xkv_arrange(k2[b]) for b in range(B)}
    xv = {b: xkv_arrange(v2[b]) for b in range(B)}

    in_maps = []
    for c in range(NCORES):
        b = c // 4
        l = c % 4
        rs = slice(CHD * l, CHD * (l + 1))
        in_maps.append({
            "xqW": xq[b],
            "xkB": xk[b],
            "xvB": xv[b],
            "wqF": w_arrange(Wq[rs, :]),
            "wkF": w_arrange(Wk[rs, :]),
            "wvF": w_arrange(Wv[rs, :]),
            "woTs": np.ascontiguousarray(Wo[:, rs].T).astype(bf16),
        })

    _LAST_IN_MAPS = in_maps
    res = run_bass_kernel_spmd(nc, in_maps, core_ids=list(range(NCORES)))
    parts = [np.asarray(res.results[c]["pout"], dtype=np.float32)
             for c in range(NCORES)]
    out = np.empty((B, S, D), dtype=np.float32)
    for b in range(B):
        out[b] = parts[4 * b] + parts[4 * b + 1] + parts[4 * b + 2] + parts[4 * b + 3]
    if debug:
        return out, res
    return out


# revision 27
# speedup vs baseline: 1.0048x; 1.0048x over previous
"""Trainium2 Bass kernel for nn_MultiHeadAttention (B=2, S=2048, D=1024, H=16).

Sharding: 8 cores = 2 batch groups x 4 cores. Core c handles batch c//4 and
heads 4*(c%4) .. 4*(c%4)+4 (CHD=256 head-dims). Each core computes Q/K/V
projections for its batch+heads, transposed-layout attention (softmax
denominators via a ones-augmented V column), and a partial output projection
over its 256 head-dims. Host sums the 4 bf16 partials per batch in f32.

All tensors bf16 (PSUM accumulation f32). Inputs are pre-arranged on the host
into [128-partition, flat] layouts so every DMA is a contiguous run per
partition (cheap descriptors). The scalar-engine exp is the pacing engine
(~147us); the scores+exp stream runs up to two head-pair groups ahead of the
attn@V stream (pt tiles buffered in SBUF) so exp also saturates during K/V
projection; Q/out projections drain as fillers inside the attention stream.
"""

from collections import deque

import numpy as np

B, S, D, H = 2, 2048, 1024, 16
HD = D // H          # 64
NCORES = 8
HPC = 4              # heads per core
CHD = HPC * HD       # 256 head-dims per core
TOK = S              # tokens per core (one batch)
QW = 512             # query window
NQW = TOK // QW      # 4 windows
NKT = TOK // 128     # 16 key tiles
CB = 256             # K/V projection token-block
NCB = TOK // CB      # 8 blocks
SCALE = 1.0 / np.sqrt(np.float32(D))  # 1/32

_PROG = None
_LAST_IN_MAPS = None


def _build(debug=False):
    from contextlib import ExitStack

    import concourse.bass as bass
    import concourse.tile as tile
    from concourse import bacc, mybir

    BF16 = mybir.dt.bfloat16
    F32 = mybir.dt.float32
    EXP = mybir.ActivationFunctionType.Exp

    nc = bacc.Bacc("TRN2", target_bir_lowering=False, debug=False,
                   num_devices=NCORES)

    # host-pre-arranged flat layouts: contiguous per-partition runs
    xqW = nc.dram_tensor("xqW", [128, NQW * 8 * QW], BF16, kind="ExternalInput").ap()
    xkB = nc.dram_tensor("xkB", [128, NCB * 8 * CB], BF16, kind="ExternalInput").ap()
    xvB = nc.dram_tensor("xvB", [128, NCB * 8 * CB], BF16, kind="ExternalInput").ap()
    wqF = nc.dram_tensor("wqF", [128, 8 * CHD], BF16, kind="ExternalInput").ap()
    wkF = nc.dram_tensor("wkF", [128, 8 * CHD], BF16, kind="ExternalInput").ap()
    wvF = nc.dram_tensor("wvF", [128, 8 * CHD], BF16, kind="ExternalInput").ap()
    woTs = nc.dram_tensor("woTs", [CHD, D], BF16, kind="ExternalInput").ap()
    pout = nc.dram_tensor("pout", [TOK, D], BF16, kind="ExternalOutput").ap()
    if debug:
        dbg = {
            "dKT": nc.dram_tensor("dKT", [2, 128, TOK], BF16, kind="ExternalOutput").ap(),
            "dvnat": nc.dram_tensor("dvnat", [128, NKT * 260], BF16, kind="ExternalOutput").ap(),
            "dqt": nc.dram_tensor("dqt", [2, 128, QW], BF16, kind="ExternalOutput").ap(),
            "dpt": nc.dram_tensor("dpt", [128, 2 * QW], BF16, kind="ExternalOutput").ap(),
            "deAB": nc.dram_tensor("deAB", [2, 65, QW], F32, kind="ExternalOutput").ap(),
            "drr": nc.dram_tensor("drr", [2, QW], F32, kind="ExternalOutput").ap(),
            "dbc": nc.dram_tensor("dbc", [2, 64, QW], F32, kind="ExternalOutput").ap(),
            "dctx": nc.dram_tensor("dctx", [2, 128, TOK], BF16, kind="ExternalOutput").ap(),
        }

    with tile.TileContext(nc) as tc, ExitStack() as ctx:
        const = ctx.enter_context(tc.tile_pool(name="const", bufs=1))
        wq_sb = const.tile([128, 8 * CHD], BF16, tag="wq")
        wk_sb = const.tile([128, 8 * CHD], BF16, tag="wk")
        wv_sb = const.tile([128, 8 * CHD], BF16, tag="wv")
        wo_sb = [const.tile([128, D], BF16, tag=f"wo{p}", name=f"wo{p}")
                 for p in range(2)]

        def wsl(t, ko, lo, hi):
            return t[:, ko * CHD + lo:ko * CHD + hi]

        # startup DMA order: K-proj deps first (wk, xbk0), then Q (wq, qx0)
        nc.sync.dma_start(out=wk_sb, in_=wkF)
        xblk = ctx.enter_context(tc.tile_pool(name="xblk", bufs=4))
        xbk0 = xblk.tile([128, 8 * CB], BF16, tag="xb", name="xbk0")
        nc.sync.dma_start(out=xbk0, in_=xkB[:, 0:8 * CB])
        nc.sync.dma_start(out=wq_sb, in_=wqF)
        qx_pool = ctx.enter_context(tc.tile_pool(name="qx", bufs=2))
        qx0 = qx_pool.tile([128, 8 * QW], BF16, tag="qx", name="qx0")
        nc.sync.dma_start(out=qx0, in_=xqW[:, 0:8 * QW])

        # warm the exp table early
        warm = const.tile([1, 8], F32)
        nc.vector.memset(warm, 0.0)
        nc.scalar.activation(out=warm, in_=warm, func=EXP)

        # persistent attention operands
        kqt = ctx.enter_context(tc.tile_pool(name="kqt", bufs=1))
        KT = [kqt.tile([128, TOK], BF16, tag=f"kt{m}", name=f"KT{m}")
              for m in range(2)]
        vnat = kqt.tile([128, NKT, HPC * (HD + 1)], BF16, tag="vnat")
        ctxP = [kqt.tile([128, TOK], BF16, tag=f"ctxP{p}", name=f"ctxP{p}")
                for p in range(2)]
        ones16 = const.tile([128, NKT], BF16)
        nc.vector.memset(ones16, 1.0)
        ones64f = const.tile([1, 64], F32, tag="ones64f")
        nc.vector.memset(ones64f, 1.0)
        ones64r = const.tile([1, 64], mybir.dt.float32r, tag="ones64r")
        nc.vector.tensor_copy(ones64r[:], ones64f[:])
        for h in range(HPC):
            nc.vector.tensor_copy(vnat[:, :, h * 65 + 64], ones16[:])

        # PSUM: sc 2x[128,1024]f32 (4 banks) + cop 2x[65,512] (2) + pp 2x[128,512] (2)
        sc_ps = ctx.enter_context(tc.tile_pool(name="sc_ps", bufs=2, space="PSUM"))
        cop = ctx.enter_context(tc.tile_pool(name="cop", bufs=2, space="PSUM"))
        pp = ctx.enter_context(tc.tile_pool(name="pp", bufs=2, space="PSUM"))

        qt_pool = ctx.enter_context(tc.tile_pool(name="qtw", bufs=4))
        pt_pool = ctx.enter_context(tc.tile_pool(name="ptp", bufs=36))
        nrm = ctx.enter_context(tc.tile_pool(name="nrm", bufs=2))
        rdp = ctx.enter_context(tc.tile_pool(name="rdp", bufs=2, space="DRAM"))
        oev = ctx.enter_context(tc.tile_pool(name="oev", bufs=3))

        qts = {}   # window -> [qt_pair0, qt_pair1];  (w, "x") -> staged qx
        fillers = deque()

        def drain(n):
            for _ in range(min(n, len(fillers))):
                fillers.popleft()()

        def flush():
            drain(len(fillers))

        # ---- Q projection (window w) as units ----
        def emit_qproj_units(w, dma_now=False):
            def u_dma():
                qxb = qx_pool.tile([128, 8 * QW], BF16, tag="qx", name=f"qx{w}")
                nc.sync.dma_start(out=qxb, in_=xqW[:, w * 8 * QW:(w + 1) * 8 * QW])
                qts[(w, "x")] = qxb

            def mk_mm(m):
                def u_mm():
                    qp = pp.tile([128, QW], F32, tag="pp", name=f"qp{w}_{m}")
                    qxb = qts[(w, "x")]
                    for ko in range(8):
                        nc.tensor.matmul(
                            qp[:], wsl(wq_sb, ko, m * 128, (m + 1) * 128),
                            qxb[:, ko * QW:(ko + 1) * QW],
                            start=(ko == 0), stop=(ko == 7))
                    qt = qt_pool.tile([128, QW], BF16, tag="qt", name=f"qt{w}_{m}")
                    nc.vector.tensor_copy(qt[:], qp[:])
                    qts.setdefault(w, [None, None])[m] = qt
                return u_mm

            if w == 0:
                qts[(0, "x")] = qx0
                for u in (mk_mm(0), mk_mm(1)):
                    u()
            elif dma_now:
                u_dma()
                fillers.extend([mk_mm(0), mk_mm(1)])
            else:
                fillers.extend([u_dma, mk_mm(0), mk_mm(1)])

        # ---- output projection (window w) as units ----
        def emit_outproj_units(w, alt_pool=False):
            q0 = w * QW
            for tt in range(QW // 128):
                t0 = q0 + tt * 128
                for et in range(2):
                    box = {}
                    use_sc = alt_pool and (tt * 2 + et) % 2 == 1

                    def u_mm(t0=t0, et=et, box=box, use_sc=use_sc):
                        if use_sc:
                            pol = sc_ps.tile([128, 2 * QW], F32, tag="sc",
                                             name=f"po{t0}_{et}")
                            po = pol[:, 0:512]
                        else:
                            po = pp.tile([128, 512], F32, tag="pp",
                                         name=f"po{t0}_{et}")
                        for p in range(2):
                            nc.tensor.matmul(
                                po[:], ctxP[p][:, t0:t0 + 128],
                                wo_sb[p][:, et * 512:(et + 1) * 512],
                                start=(p == 0), stop=(p == 1))
                        box["po"] = po

                    def u_ev(t0=t0, et=et, box=box):
                        ot = oev.tile([128, 512], BF16, tag="ot")
                        nc.vector.tensor_copy(ot[:], box["po"][:])
                        nc.sync.dma_start(
                            out=pout[t0:t0 + 128, et * 512:(et + 1) * 512],
                            in_=ot[:])

                    fillers.append(u_mm)
                    fillers.append(u_ev)

        # ---- attention pieces ----
        def scores_exp(w, p, kt):
            qtp = qts[w][p]
            sc = sc_ps.tile([128, 2 * QW], F32, tag="sc", name=f"sc{w}_{p}_{kt}")
            nc.tensor.matmul(
                sc[:, 0:QW], KT[p][0:64, kt * 128:(kt + 1) * 128],
                qtp[0:64, :], start=True, stop=True, tile_position=(0, 0))
            nc.tensor.matmul(
                sc[:, QW:2 * QW], KT[p][64:128, kt * 128:(kt + 1) * 128],
                qtp[64:128, :], start=True, stop=True, tile_position=(64, 0))
            pt = pt_pool.tile([128, 2 * QW], BF16, tag="pt")
            nc.scalar.activation(out=pt[:], in_=sc[:], func=EXP,
                                 scale=float(SCALE))
            if debug and (w, p, kt) == (0, 0, 0):
                nc.sync.dma_start(out=dbg["dpt"], in_=pt[:])
            return pt

        def attn_v(p, kt, pt, cA, cB):
            hA, hB = 2 * p, 2 * p + 1
            nc.tensor.matmul(
                cA[:], vnat[:, kt, hA * 65:hA * 65 + 65], pt[:, 0:QW],
                start=(kt == 0), stop=(kt == NKT - 1))
            nc.tensor.matmul(
                cB[:], vnat[:, kt, hB * 65:hB * 65 + 65], pt[:, QW:2 * QW],
                start=(kt == 0), stop=(kt == NKT - 1))

        # ---- softmax normalize: part A frees PSUM accumulators fast; ----
        # ---- part B (DRAM-broadcast + muls) runs later as a filler. ----
        def normalize_a(w, p, cA, cB):
            eA = nrm.tile([65, QW], F32, tag="eA")
            eB = nrm.tile([65, QW], F32, tag="eB")
            nc.vector.tensor_copy(eA[:], cA[:])
            nc.vector.tensor_copy(eB[:], cB[:])
            rsA = nrm.tile([1, QW], F32, tag="rsA")
            rsB = nrm.tile([1, QW], F32, tag="rsB")
            nc.vector.tensor_copy(rsA[:], eA[64:65, :])
            nc.vector.tensor_copy(rsB[:], eB[64:65, :])
            rrA = nrm.tile([1, QW], F32, tag="rrA")
            rrB = nrm.tile([1, QW], F32, tag="rrB")
            nc.vector.reciprocal_approx_fast(rrA[:], rsA[:])
            nc.vector.reciprocal_approx_fast(rrB[:], rsB[:])
            if debug and (w, p) == (0, 0):
                nc.sync.dma_start(out=dbg["deAB"][0], in_=eA[:])
                nc.sync.dma_start(out=dbg["deAB"][1], in_=eB[:])
                nc.sync.dma_start(out=dbg["drr"][0:1], in_=rrA[:])
                nc.sync.dma_start(out=dbg["drr"][1:2], in_=rrB[:])
            return eA, eB, rrA, rrB

        def normalize_b(w, p, eA, eB, rrA, rrB):
            q0 = w * QW
            rden = rdp.tile([2, QW], F32, tag="rden")
            nc.gpsimd.dma_start(out=rden[0:1, :], in_=rrA[:])
            nc.gpsimd.dma_start(out=rden[1:2, :], in_=rrB[:])
            bcA = nrm.tile([64, QW], F32, tag="bcA")
            bcB = nrm.tile([64, QW], F32, tag="bcB")
            nc.gpsimd.dma_start(
                out=bcA, in_=bass.AP(tensor=rden.tensor, offset=rden.offset,
                                     ap=[[0, 64], [1, QW]]))
            nc.gpsimd.dma_start(
                out=bcB, in_=bass.AP(tensor=rden.tensor,
                                     offset=rden.offset + QW,
                                     ap=[[0, 64], [1, QW]]))
            if debug and (w, p) == (0, 0):
                nc.sync.dma_start(out=dbg["dbc"][0], in_=bcA[:])
                nc.sync.dma_start(out=dbg["dbc"][1], in_=bcB[:])
            nc.vector.tensor_mul(ctxP[p][0:64, q0:q0 + QW], eA[0:64, :],
                                 bcA[:])
            scb = nrm.tile([64, QW], BF16, tag="scb")
            nc.vector.tensor_mul(scb[:], eB[0:64, :], bcB[:])
            nc.gpsimd.dma_start(out=ctxP[p][64:128, q0:q0 + QW], in_=scb[:])

        def normalize_b_pe(w, p, eA, eB, rrA, rrB):
            q0 = w * QW
            rrAr = nrm.tile([1, QW], mybir.dt.float32r, tag="rrAr")
            rrBr = nrm.tile([1, QW], mybir.dt.float32r, tag="rrBr")
            nc.vector.tensor_copy(rrAr[:], rrA[:])
            nc.vector.tensor_copy(rrBr[:], rrB[:])
            bc = sc_ps.tile([128, 2 * QW], F32, tag="sc", name=f"bcp{w}_{p}")
            nc.tensor.matmul(bc[0:64, 0:QW], ones64r[:], rrAr[:],
                             start=True, stop=True)
            nc.tensor.matmul(bc[0:64, QW:2 * QW], ones64r[:], rrBr[:],
                             start=True, stop=True)
            nc.vector.tensor_mul(ctxP[p][0:64, q0:q0 + QW], eA[0:64, :],
                                 bc[0:64, 0:QW])
            scb = nrm.tile([64, QW], BF16, tag="scb")
            nc.vector.tensor_mul(scb[:], eB[0:64, :], bc[0:64, QW:2 * QW])
            nc.gpsimd.dma_start(out=ctxP[p][64:128, q0:q0 + QW], in_=scb[:])

        # ---------------- schedule ----------------
        # PE warm-keeper: dependency-free tiny matmuls run while the first
        # input DMAs stream in, keeping the HAM clock-gate at full rate.
        dwp = pp.tile([128, 512], F32, tag="pp", name="dwp")
        for i in range(150):
            nc.tensor.matmul(dwp[0:16, 0:16], ones16[:, 0:16], ones16[:, 0:16],
                             start=True, stop=True)
        emit_qproj_units(0)

        grp = [(w, p) for w in range(NQW) for p in range(2)]
        pt_store = {}          # (group_idx, kt) -> pt tile

        attn_state = {"g": 0, "kt": 0, "cA": None, "cB": None}

        def emit_attn_step():
            g, kt = attn_state["g"], attn_state["kt"]
            if g >= len(grp) or (g, kt) not in pt_store:
                return False
            w, p = grp[g]
            if kt == 0:
                attn_state["cA"] = cop.tile([65, QW], F32, tag="ctx",
                                            name=f"cA_{w}_{p}")
                attn_state["cB"] = cop.tile([65, QW], F32, tag="ctx",
                                            name=f"cB_{w}_{p}")
            pt = pt_store.pop((g, kt))
            attn_v(p, kt, pt, attn_state["cA"], attn_state["cB"])
            if kt == NKT - 1:
                ns = normalize_a(w, p, attn_state["cA"], attn_state["cB"])
                if g == len(grp) - 1:
                    dwp2 = pp.tile([128, 512], F32, tag="pp", name="dwp2")
                    for _ in range(50):
                        nc.tensor.matmul(dwp2[0:16, 0:16], ones16[:, 0:16],
                                         ones16[:, 0:16], start=True, stop=True)
                    normalize_b_pe(w, p, *ns)
                    for _ in range(60):
                        nc.tensor.matmul(dwp2[0:16, 0:16], ones16[:, 0:16],
                                         ones16[:, 0:16], start=True, stop=True)
                else:
                    fillers.append(
                        lambda w=w, p=p, ns=ns: normalize_b(w, p, *ns))
                if p == 1:
                    emit_outproj_units(w, alt_pool=(w == NQW - 1))
                attn_state["g"] += 1
                attn_state["kt"] = 0
            else:
                attn_state["kt"] = kt + 1
            return True

        # ---- phase 0: K/V projection blocks + scores/exp for window-0 ----
        for blk in range(NCB):
            if blk == 5:
                emit_qproj_units(1)
            c0 = blk * CB
            if blk > 0:
                xbk = xblk.tile([128, 8 * CB], BF16, tag="xb", name=f"xbk{blk}")
                nc.sync.dma_start(out=xbk,
                                  in_=xkB[:, blk * 8 * CB:(blk + 1) * 8 * CB])
            else:
                xbk = xbk0
                nc.sync.dma_start(out=wv_sb, in_=wvF)
            xbv = xblk.tile([128, 8 * CB], BF16, tag="xb", name=f"xbv{blk}")
            nc.sync.dma_start(out=xbv,
                              in_=xvB[:, blk * 8 * CB:(blk + 1) * 8 * CB])
            psk = pp.tile([128, 2 * CB], F32, tag="pp", name=f"psk{blk}")
            for m in range(2):
                for ko in range(8):
                    nc.tensor.matmul(
                        psk[:, m * CB:(m + 1) * CB],
                        wsl(wk_sb, ko, m * 128, (m + 1) * 128),
                        xbk[:, ko * CB:(ko + 1) * CB],
                        start=(ko == 0), stop=(ko == 7))
            for m in range(2):
                nc.vector.tensor_copy(
                    KT[m][:, c0:c0 + CB], psk[:, m * CB:(m + 1) * CB])
            # scores+exp for both window-0 head pairs on this block's key
            # tiles; V-proj matmuls cover the exp latency.
            for kt in (2 * blk, 2 * blk + 1):
                pt_store[(0, kt)] = scores_exp(0, 0, kt)
            for tt in range(2):
                bi = 2 * blk + tt
                pv = pp.tile([128, CHD], F32, tag="pp", name=f"pv{bi}")
                for ko in range(8):
                    nc.tensor.matmul(
                        pv[:], xbv[:, ko * CB + tt * 128:ko * CB + (tt + 1) * 128],
                        wsl(wv_sb, ko, 0, CHD), start=(ko == 0), stop=(ko == 7))
                nc.vector.tensor_copy(
                    vnat[:, bi, :].rearrange("p (h c) -> p h c", c=65)[:, :, 0:64],
                    pv[:].rearrange("p (h c) -> p h c", c=64))
            for kt in (2 * blk, 2 * blk + 1):
                pt_store[(1, kt)] = scores_exp(0, 1, kt)
            drain(2)

        for p_ in range(2):
            nc.sync.dma_start(out=wo_sb[p_],
                              in_=woTs[p_ * 128:(p_ + 1) * 128, :])
        # ---- phase 1: exp stream 2 groups ahead; attn catches up ----
        for eg in range(2, len(grp) + 2):
            if eg < len(grp):
                w, p = grp[eg]
                if p == 0 and w + 1 < NQW:
                    emit_qproj_units(w + 1)
            for kt in range(NKT):
                if eg < len(grp):
                    pt_store[(eg, kt)] = scores_exp(grp[eg][0], grp[eg][1], kt)
                nsteps = 2 if len(pt_store) > 3 else 1
                for _ in range(nsteps):
                    emit_attn_step()
                # near attn-group boundaries keep the vector queue shallow so
                # the next group's PSUM accumulators free up promptly
                akt = attn_state["kt"]
                endgame = (attn_state["g"] == len(grp) - 1 and akt > 8)
                if 2 <= akt <= 12 and not endgame:
                    drain(1 if len(fillers) < 12 else 2)
        while emit_attn_step():
            if attn_state["g"] < len(grp) - 1 or attn_state["kt"] <= 8:
                drain(1)
        flush()
        if debug:
            for m in range(2):
                nc.sync.dma_start(out=dbg["dKT"][m], in_=KT[m][:])
            nc.sync.dma_start(
                out=dbg["dvnat"],
                in_=vnat[:].rearrange("p a b -> p (a b)"))
            for p_ in range(2):
                nc.sync.dma_start(out=dbg["dqt"][p_], in_=qts[0][p_][:])
                nc.sync.dma_start(out=dbg["dctx"][p_], in_=ctxP[p_][:])

    nc.compile()
    return nc


def kernel(query, key, value, Wq, Wk, Wv, Wo, debug=False):
    global _PROG, _LAST_IN_MAPS
    import ml_dtypes
    from concourse.bass_utils import run_bass_kernel_spmd

    if _PROG is None or debug:
        _PROG = _build(debug=debug)
    nc = _PROG
    if debug:
        _PROG = None

    bf16 = ml_dtypes.bfloat16
    q2 = np.asarray(query, dtype=np.float32).reshape(B, S, D)
    k2 = np.asarray(key, dtype=np.float32).reshape(B, S, D)
    v2 = np.asarray(value, dtype=np.float32).reshape(B, S, D)
    Wq = np.asarray(Wq, dtype=np.float32)
    Wk = np.asarray(Wk, dtype=np.float32)
    Wv = np.asarray(Wv, dtype=np.float32)
    Wo = np.asarray(Wo, dtype=np.float32)

    def xq_arrange(x):  # [S, D] -> [128, NQW*8*QW] window-major
        a = x.T.reshape(8, 128, S).transpose(1, 0, 2)          # ki, ko, t
        a = a.reshape(128, 8, NQW, QW).transpose(0, 2, 1, 3)   # ki, w, ko, t
        return np.ascontiguousarray(a.reshape(128, NQW * 8 * QW)).astype(bf16)

    def xkv_arrange(x):  # [S, D] -> [128, NCB*8*CB] block-major
        a = x.T.reshape(8, 128, S).transpose(1, 0, 2)
        a = a.reshape(128, 8, NCB, CB).transpose(0, 2, 1, 3)   # ki, blk, ko, t
        return np.ascontiguousarray(a.reshape(128, NCB * 8 * CB)).astype(bf16)

    def w_arrange(Wsl):  # [CHD, D] slice -> [128, 8*CHD]
        a = Wsl.T.reshape(8, 128, CHD).transpose(1, 0, 2)      # ki, ko, m
        return np.ascontiguousarray(a.reshape(128, 8 * CHD)).astype(bf16)

    xq = {b: xq_arrange(q2[b]) for b in range(B)}
    xk = {b: xkv_arrange(k2[b]) for b in range(B)}
    xv = {b: xkv_arrange(v2[b]) for b in range(B)}

    in_maps = []
    for c in range(NCORES):
        b = c // 4
        l = c % 4
        rs = slice(CHD * l, CHD * (l + 1))
        in_maps.append({
            "xqW": xq[b],
            "xkB": xk[b],
            "xvB": xv[b],
            "wqF": w_arrange(Wq[rs, :]),
            "wkF": w_arrange(Wk[rs, :]),
            "wvF": w_arrange(Wv[rs, :]),
            "woTs": np.ascontiguousarray(Wo[:, rs].T).astype(bf16),
        })

    _LAST_IN_MAPS = in_maps
    res = run_bass_kernel_spmd(nc, in_maps, core_ids=list(range(NCORES)))
    parts = [np.asarray(res.results[c]["pout"], dtype=np.float32)
             for c in range(NCORES)]
    out = np.empty((B, S, D), dtype=np.float32)
    for b in range(B):
        out[b] = parts[4 * b] + parts[4 * b + 1] + parts[4 * b + 2] + parts[4 * b + 3]
    if debug:
        return out, res
    return out
